# revision 13
# baseline (speedup 1.0000x reference)
"""MultiHeadDecoder (moe_routing) Trainium2 kernel.

Strategy: expert-parallel. Each of the 8 cores owns one head's weights.
Host groups samples by head index, pads each group to a common capacity C,
and transposes X so the contraction dim lands on partitions. Each core runs
a dense 2-layer MLP (256->512 relu, 512->2048) for its head's samples.
Host scatters rows back to original order.

Layer 1 computes H^T (hid on partitions) so layer 2 can contract over hid
without an on-chip transpose:
  H^T[hc]  = W1[:, hc].T @ X^T      (lhsT=W1 chunk, rhs=X^T chunk)
  out[st]  = (H^T[:, st]).T @ W2    (lhsT=H^T chunk, rhs=W2 chunk)

Matmuls run in float32r (fp32 bits, full PE rate, tf32-ish multiply).
Inputs are packed host-side into the exact SBUF layout so every DMA has
long contiguous runs per partition. All inputs stream on the sync (SP)
HWDGE ring; all output stores go on the scalar (Act) ring so they never
queue behind the 4MB W2 stream (rings are FIFO). Stage B is ordered
oc-outer so only the first W2 chunk's DMA gates its start.
"""

import numpy as np

import concourse.bass as bass
import concourse.mybir as mybir
from concourse import bacc
from concourse.tile import TileContext
from concourse.bass_utils import run_bass_kernel_spmd

IN_F, HID, OUT_F, N_HEADS, BATCH = 256, 512, 2048, 8, 4096
N_CORES = 8
P = 128
KI = IN_F // P     # 2  input-feature chunks
HC = HID // P      # 4  hidden chunks
OC = OUT_F // 512  # 4  output-feature chunks of 512

f32 = mybir.dt.float32
f32r = mybir.dt.float32r  # fp32 bits, PE runs at full (bf16) rate, tf32-ish mul

_NC_CACHE: dict = {}


def build_nc(C: int):
    """Build the per-core Bass program for capacity C (multiple of 128)."""
    ST = C // P      # sample tiles
    KF = C + HID     # free size of one k-part: xt_k then w1_k

    nc = bacc.Bacc("TRN2", target_bir_lowering=False, debug=False,
                   num_devices=N_CORES)
    xin = nc.dram_tensor("xin", [KI, P, KF], f32r, kind="ExternalInput")
    b1s = nc.dram_tensor("b1s", [P, HC], f32, kind="ExternalInput")
    w2p = nc.dram_tensor("w2p", [OC, P, HC * 512], f32r, kind="ExternalInput")
    b2 = nc.dram_tensor("b2", [1, OUT_F], f32, kind="ExternalInput")
    out = nc.dram_tensor("out", [C, OUT_F], f32, kind="ExternalOutput")

    relu = mybir.ActivationFunctionType.Relu

    with TileContext(nc) as tc:
        with (
            tc.tile_pool(name="const", bufs=1) as const,
            tc.tile_pool(name="psumA", bufs=2, space="PSUM") as psumA,
            tc.tile_pool(name="psumB", bufs=5, space="PSUM") as psumB,
            tc.tile_pool(name="psumW", bufs=1, space="PSUM") as psumW,
            tc.tile_pool(name="outp", bufs=6) as outp,
        ):
            # HAM warmup: dummy matmuls with no DMA deps keep the PE busy
            # while inputs stream in, so real matmuls run at 2.4 GHz.
            wsrc = const.tile([P, 256], f32, tag="warm")
            nc.vector.memset(wsrc[:], 0.0)
            wps = psumW.tile([P, 256], f32, tag="warmps")
            for _ in range(30):
                nc.tensor.matmul(wps[:], lhsT=wsrc[:, :P], rhs=wsrc[:],
                                 start=True, stop=True)
            # Stage-A inputs first so the PE starts ASAP; W2 streams behind.
            # Two k-part DMAs so the k=0 matmuls can start at half-arrival.
            xin_ks = []
            for k in range(KI):
                xk = const.tile([P, KF], f32r, tag=f"xin_{k}")
                nc.sync.dma_start(xk[:], xin[k])
                xin_ks.append(xk)
            b1_s = const.tile([P, HC], f32)
            nc.sync.dma_start(b1_s[:], b1s[:])
            b2_row = const.tile([1, OUT_F], f32)
            nc.sync.dma_start(b2_row[:], b2[:])
            b2_s = const.tile([P, OUT_F], f32)
            nc.gpsimd.partition_broadcast(b2_s[:], b2_row[:])
            w2_cs = []
            for oc in range(OC):
                w2_c = const.tile([P, HC * 512], f32r, tag=f"w2_{oc}")
                nc.sync.dma_start(w2_c[:], w2p[oc])
                w2_cs.append(w2_c)

            # Stage A: H^T [hid(part), sample(free)], relu(x @ W1 + b1)
            # sgroups outer so stage B's early sample tiles are ready sooner.
            ht = const.tile([P, HC, C], f32r)
            sgroups = [(s, min(512, C - s)) for s in range(0, C, 512)]
            for (s0, sn) in sgroups:
                for hc in range(HC):
                    ps = psumA.tile([P, 512], f32, tag="psA")
                    for k in range(KI):
                        nc.tensor.matmul(
                            ps[:, :sn],
                            lhsT=xin_ks[k][:, C + hc * P: C + (hc + 1) * P],
                            rhs=xin_ks[k][:, s0:s0 + sn],
                            start=(k == 0), stop=(k == KI - 1),
                        )
                    nc.scalar.activation(
                        ht[:, hc, s0:s0 + sn], ps[:, :sn], relu,
                        bias=b1_s[:, hc:hc + 1],
                    )

            # Stage B: out[st, oc] = H[st] @ W2[:, oc] + b2[oc]
            for oc in range(OC):
                for st in range(ST):
                    ps = psumB.tile([P, 512], f32, tag="psB")
                    for hc in range(HC):
                        nc.tensor.matmul(
                            ps[:],
                            lhsT=ht[:, hc, st * P:(st + 1) * P],
                            rhs=w2_cs[oc][:, hc * 512:(hc + 1) * 512],
                            start=(hc == 0), stop=(hc == HC - 1),
                        )
                    ot = outp.tile([P, 512], f32, tag="ot")
                    nc.vector.tensor_add(
                        out=ot[:],
                        in0=ps[:],
                        in1=b2_s[:, oc * 512:(oc + 1) * 512],
                    )
                    nc.scalar.dma_start(
                        out[st * P:(st + 1) * P, oc * 512:(oc + 1) * 512], ot[:]
                    )

    nc.compile()
    return nc


def kernel(X, X_head_idx, W1, b1, W2, b2):
    X = np.ascontiguousarray(np.asarray(X, dtype=np.float32))
    idx = np.asarray(X_head_idx).astype(np.int64)
    W1 = np.asarray(W1, dtype=np.float32)
    b1 = np.asarray(b1, dtype=np.float32)
    W2 = np.asarray(W2, dtype=np.float32)
    b2 = np.asarray(b2, dtype=np.float32)

    batch = X.shape[0]
    counts = np.bincount(idx, minlength=N_HEADS)
    order = np.argsort(idx, kind="stable")
    positions = np.split(order, np.cumsum(counts)[:-1])

    C = max(512, int(-(-counts.max() // P)) * P)
    if C not in _NC_CACHE:
        _NC_CACHE[C] = build_nc(C)
    nc = _NC_CACHE[C]

    in_maps = []
    for h in range(N_HEADS):
        pos = positions[h]
        # xin[k, p, :] = [ X[pos, k*128+p] (len C, padded) | W1[h, k*128+p, :] ]
        xin = np.zeros((KI, P, C + HID), dtype=np.float32)
        if len(pos):
            xk = X[pos].T.reshape(KI, P, len(pos))          # [k, p, c]
            xin[:, :, :len(pos)] = xk
        xin[:, :, C:] = W1[h].reshape(KI, P, HID)
        # w2 packed: [oc, p, hc*512 + o'] = W2[h, hc*128 + p, oc*512 + o']
        w2t = np.transpose(W2[h].reshape(HC, P, OUT_F), (1, 0, 2))  # [p, hc, of]
        w2p = np.empty((OC, P, HC * 512), dtype=np.float32)
        for oc in range(OC):
            w2p[oc] = w2t[:, :, oc * 512:(oc + 1) * 512].reshape(P, HC * 512)
        in_maps.append({
            "xin": xin,
            "b1s": np.ascontiguousarray(b1[h].reshape(HC, P).T),
            "w2p": w2p,
            "b2": np.ascontiguousarray(b2[h][None, :]),
        })

    res = run_bass_kernel_spmd(nc, in_maps, list(range(N_CORES)))

    out = np.empty((batch, OUT_F), dtype=np.float32)
    for h in range(N_HEADS):
        pos = positions[h]
        if len(pos):
            out[pos] = res.results[h]["out"][:len(pos)]
    return out


# revision 15
# speedup vs baseline: 1.0716x; 1.0716x over previous
"""MultiHeadDecoder (moe_routing) Trainium2 kernel.

Strategy: expert-parallel. Each of the 8 cores owns one head's weights.
Host groups samples by head index, pads each group to a common capacity C,
and transposes X so the contraction dim lands on partitions. Each core runs
a dense 2-layer MLP (256->512 relu, 512->2048) for its head's samples.
Host scatters rows back to original order.

Layer 1 computes H^T (hid on partitions) so layer 2 can contract over hid
without an on-chip transpose:
  H^T[hc]  = W1[:, hc].T @ X^T      (lhsT=W1 chunk, rhs=X^T chunk)
  out[st]  = (H^T[:, st]).T @ W2    (lhsT=H^T chunk, rhs=W2 chunk)

Matmuls run in float32r (fp32 bits, full PE rate, tf32-ish multiply).
Inputs are packed host-side into the exact SBUF layout so every DMA has
long contiguous runs per partition. All inputs stream on the sync (SP)
HWDGE ring; all output stores go on the scalar (Act) ring so they never
queue behind the 4MB W2 stream (rings are FIFO). Stage B is ordered
oc-outer so only the first W2 chunk's DMA gates its start.
"""

import numpy as np

import concourse.bass as bass
import concourse.mybir as mybir
from concourse import bacc
from concourse.tile import TileContext
from concourse.bass_utils import run_bass_kernel_spmd

IN_F, HID, OUT_F, N_HEADS, BATCH = 256, 512, 2048, 8, 4096
N_CORES = 8
P = 128
KI = IN_F // P     # 2  input-feature chunks
HC = HID // P      # 4  hidden chunks
OC = OUT_F // 512  # 4  output-feature chunks of 512

f32 = mybir.dt.float32
f32r = mybir.dt.float32r  # fp32 bits, PE runs at full (bf16) rate, tf32-ish mul

_NC_CACHE: dict = {}


def build_nc(C: int):
    """Build the per-core Bass program for capacity C (multiple of 128)."""
    ST = C // P      # sample tiles
    KF = C + HID     # free size of one k-part: xt_k then w1_k

    nc = bacc.Bacc("TRN2", target_bir_lowering=False, debug=False,
                   num_devices=N_CORES)
    xin = nc.dram_tensor("xin", [KI, P, KF], f32r, kind="ExternalInput")
    b1s = nc.dram_tensor("b1s", [P, HC], f32, kind="ExternalInput")
    w2p = nc.dram_tensor("w2p", [OC, P, HC * 512], f32r, kind="ExternalInput")
    b2 = nc.dram_tensor("b2", [1, OUT_F], f32, kind="ExternalInput")
    out = nc.dram_tensor("out", [C, OUT_F], f32, kind="ExternalOutput")

    relu = mybir.ActivationFunctionType.Relu

    with TileContext(nc) as tc:
        with (
            tc.tile_pool(name="const", bufs=1) as const,
            tc.tile_pool(name="psumA", bufs=2, space="PSUM") as psumA,
            tc.tile_pool(name="psumB", bufs=5, space="PSUM") as psumB,
            tc.tile_pool(name="psumW", bufs=1, space="PSUM") as psumW,
            tc.tile_pool(name="outp", bufs=6) as outp,
        ):
            # HAM warmup: dummy matmuls with no DMA deps keep the PE busy
            # while inputs stream in, so real matmuls run at 2.4 GHz.
            wsrc = const.tile([P, 64], f32, tag="warm")
            nc.vector.memset(wsrc[:], 0.0)
            wps = psumW.tile([64, 64], f32, tag="warmps")
            for _ in range(30):
                nc.tensor.matmul(wps[:], lhsT=wsrc[:, :64], rhs=wsrc[:],
                                 start=True, stop=True)
            # Stage-A inputs first so the PE starts ASAP; W2 streams behind.
            # Two k-part DMAs so the k=0 matmuls can start at half-arrival.
            xin_ks = []
            for k in range(KI):
                xk = const.tile([P, KF], f32r, tag=f"xin_{k}")
                nc.sync.dma_start(xk[:], xin[k])
                xin_ks.append(xk)
            b1_s = const.tile([P, HC], f32)
            nc.sync.dma_start(b1_s[:], b1s[:])
            b2_row = const.tile([1, OUT_F], f32)
            nc.sync.dma_start(b2_row[:], b2[:])
            b2_s = const.tile([P, OUT_F], f32)
            nc.gpsimd.partition_broadcast(b2_s[:], b2_row[:])
            w2_cs = []
            for oc in range(OC):
                w2_c = const.tile([P, HC * 512], f32r, tag=f"w2_{oc}")
                nc.sync.dma_start(w2_c[:], w2p[oc])
                w2_cs.append(w2_c)

            # Stage A: H^T [hid(part), sample(free)], relu(x @ W1 + b1)
            # sgroups outer so stage B's early sample tiles are ready sooner.
            ht = const.tile([P, HC, C], f32r)
            sgroups = [(s, min(512, C - s)) for s in range(0, C, 512)]
            for (s0, sn) in sgroups:
                for hc in range(HC):
                    ps = psumA.tile([P, 512], f32, tag="psA")
                    for k in range(KI):
                        nc.tensor.matmul(
                            ps[:, :sn],
                            lhsT=xin_ks[k][:, C + hc * P: C + (hc + 1) * P],
                            rhs=xin_ks[k][:, s0:s0 + sn],
                            start=(k == 0), stop=(k == KI - 1),
                        )
                    nc.scalar.activation(
                        ht[:, hc, s0:s0 + sn], ps[:, :sn], relu,
                        bias=b1_s[:, hc:hc + 1],
                    )

            # Stage B: out[st, oc] = H[st] @ W2[:, oc] + b2[oc]
            for oc in range(OC):
                for st in range(ST):
                    ps = psumB.tile([P, 512], f32, tag="psB")
                    for hc in range(HC):
                        nc.tensor.matmul(
                            ps[:],
                            lhsT=ht[:, hc, st * P:(st + 1) * P],
                            rhs=w2_cs[oc][:, hc * 512:(hc + 1) * 512],
                            start=(hc == 0), stop=(hc == HC - 1),
                        )
                    ot = outp.tile([P, 512], f32, tag="ot")
                    nc.vector.tensor_add(
                        out=ot[:],
                        in0=ps[:],
                        in1=b2_s[:, oc * 512:(oc + 1) * 512],
                    )
                    nc.scalar.dma_start(
                        out[st * P:(st + 1) * P, oc * 512:(oc + 1) * 512], ot[:]
                    )

    nc.compile()
    return nc


def kernel(X, X_head_idx, W1, b1, W2, b2):
    X = np.ascontiguousarray(np.asarray(X, dtype=np.float32))
    idx = np.asarray(X_head_idx).astype(np.int64)
    W1 = np.asarray(W1, dtype=np.float32)
    b1 = np.asarray(b1, dtype=np.float32)
    W2 = np.asarray(W2, dtype=np.float32)
    b2 = np.asarray(b2, dtype=np.float32)

    batch = X.shape[0]
    counts = np.bincount(idx, minlength=N_HEADS)
    order = np.argsort(idx, kind="stable")
    positions = np.split(order, np.cumsum(counts)[:-1])

    C = max(512, int(-(-counts.max() // P)) * P)
    if C not in _NC_CACHE:
        _NC_CACHE[C] = build_nc(C)
    nc = _NC_CACHE[C]

    in_maps = []
    for h in range(N_HEADS):
        pos = positions[h]
        # xin[k, p, :] = [ X[pos, k*128+p] (len C, padded) | W1[h, k*128+p, :] ]
        xin = np.zeros((KI, P, C + HID), dtype=np.float32)
        if len(pos):
            xk = X[pos].T.reshape(KI, P, len(pos))          # [k, p, c]
            xin[:, :, :len(pos)] = xk
        xin[:, :, C:] = W1[h].reshape(KI, P, HID)
        # w2 packed: [oc, p, hc*512 + o'] = W2[h, hc*128 + p, oc*512 + o']
        w2t = np.transpose(W2[h].reshape(HC, P, OUT_F), (1, 0, 2))  # [p, hc, of]
        w2p = np.empty((OC, P, HC * 512), dtype=np.float32)
        for oc in range(OC):
            w2p[oc] = w2t[:, :, oc * 512:(oc + 1) * 512].reshape(P, HC * 512)
        in_maps.append({
            "xin": xin,
            "b1s": np.ascontiguousarray(b1[h].reshape(HC, P).T),
            "w2p": w2p,
            "b2": np.ascontiguousarray(b2[h][None, :]),
        })

    res = run_bass_kernel_spmd(nc, in_maps, list(range(N_CORES)))

    out = np.empty((batch, OUT_F), dtype=np.float32)
    for h in range(N_HEADS):
        pos = positions[h]
        if len(pos):
            out[pos] = res.results[h]["out"][:len(pos)]
    return out


# revision 16
# speedup vs baseline: 1.1171x; 1.0425x over previous
"""MultiHeadDecoder (moe_routing) Trainium2 kernel.

Strategy: expert-parallel. Each of the 8 cores owns one head's weights.
Host groups samples by head index, pads each group to a common capacity C
(multiple of 64), and transposes X so the contraction dim lands on
partitions. Each core runs a dense 2-layer MLP (256->512 relu, 512->2048)
for its head's samples. Host scatters rows back to original order.

Layer 1 computes H^T (hid on partitions) so layer 2 can contract over hid
without an on-chip transpose:
  H^T[hc]  = W1[:, hc].T @ X^T      (lhsT=W1 chunk, rhs=X^T chunk)
  out[st]  = (H^T[:, st]).T @ W2    (lhsT=H^T chunk, rhs=W2 chunk)

Matmuls run in float32r (fp32 bits, full PE rate, tf32-ish multiply).
Inputs are packed host-side into the exact SBUF layout so every DMA has
long contiguous runs per partition. All inputs stream on the sync (SP)
HWDGE ring; all output stores go on the scalar (Act) ring so they never
queue behind the W2 stream (rings are FIFO per issuing engine). Stage B
is ordered oc-outer so only the first W2 chunk's DMA gates its start.
Dummy matmuls keep the PE's HAM clock-gate warm while DMAs stream.
"""

import numpy as np

import concourse.bass as bass
import concourse.mybir as mybir
from concourse import bacc
from concourse.tile import TileContext
from concourse.bass_utils import run_bass_kernel_spmd

IN_F, HID, OUT_F, N_HEADS, BATCH = 256, 512, 2048, 8, 4096
N_CORES = 8
P = 128
KI = IN_F // P     # 2  input-feature chunks
HC = HID // P      # 4  hidden chunks
OC = OUT_F // 512  # 4  output-feature chunks of 512

f32 = mybir.dt.float32
f32r = mybir.dt.float32r  # fp32 bits, PE runs at full (bf16) rate, tf32-ish mul

_NC_CACHE: dict = {}


def build_nc(C: int):
    """Build the per-core Bass program for capacity C (multiple of 64)."""
    KF = C + HID     # free size of one k-part: xt_k then w1_k
    stiles = [(s, min(P, C - s)) for s in range(0, C, P)]
    sgroups = [(s, min(512, C - s)) for s in range(0, C, 512)]

    nc = bacc.Bacc("TRN2", target_bir_lowering=False, debug=False,
                   num_devices=N_CORES)
    xin = nc.dram_tensor("xin", [KI, P, KF], f32r, kind="ExternalInput")
    b1s = nc.dram_tensor("b1s", [P, HC], f32, kind="ExternalInput")
    w2p = nc.dram_tensor("w2p", [OC, P, HC * 512], f32r, kind="ExternalInput")
    b2 = nc.dram_tensor("b2", [1, OUT_F], f32, kind="ExternalInput")
    out = nc.dram_tensor("out", [C, OUT_F], f32, kind="ExternalOutput")

    relu = mybir.ActivationFunctionType.Relu

    with TileContext(nc) as tc:
        with (
            tc.tile_pool(name="const", bufs=1) as const,
            tc.tile_pool(name="psumA", bufs=2, space="PSUM") as psumA,
            tc.tile_pool(name="psumB", bufs=5, space="PSUM") as psumB,
            tc.tile_pool(name="psumW", bufs=1, space="PSUM") as psumW,
            tc.tile_pool(name="outp", bufs=6) as outp,
        ):
            # HAM warmup: dummy matmuls with no DMA deps keep the PE busy
            # while inputs stream in, so real matmuls run at 2.4 GHz.
            wsrc = const.tile([P, 64], f32, tag="warm")
            nc.vector.memset(wsrc[:], 0.0)
            wps = psumW.tile([64, 64], f32, tag="warmps")
            for _ in range(35):
                nc.tensor.matmul(wps[:], lhsT=wsrc[:, :64], rhs=wsrc[:],
                                 start=True, stop=True)

            # Stage-A inputs first so the PE starts ASAP; W2 streams behind.
            # Two k-part DMAs so the k=0 matmuls can start at half-arrival.
            xin_ks = []
            for k in range(KI):
                xk = const.tile([P, KF], f32r, tag=f"xin_{k}")
                nc.sync.dma_start(xk[:], xin[k])
                xin_ks.append(xk)
            b1_s = const.tile([P, HC], f32)
            nc.sync.dma_start(b1_s[:], b1s[:])
            b2_row = const.tile([1, OUT_F], f32)
            nc.sync.dma_start(b2_row[:], b2[:])
            b2_s = const.tile([P, OUT_F], f32)
            nc.gpsimd.partition_broadcast(b2_s[:], b2_row[:])
            w2_cs = []
            for oc in range(OC):
                w2_c = const.tile([P, HC * 512], f32r, tag=f"w2_{oc}")
                nc.sync.dma_start(w2_c[:], w2p[oc])
                w2_cs.append(w2_c)

            # Stage A: H^T [hid(part), sample(free)], relu(x @ W1 + b1)
            # sgroups outer so stage B's early sample tiles are ready sooner.
            ht = const.tile([P, HC, C], f32r)
            for (s0, sn) in sgroups:
                for hc in range(HC):
                    ps = psumA.tile([P, 512], f32, tag="psA")
                    for k in range(KI):
                        nc.tensor.matmul(
                            ps[:, :sn],
                            lhsT=xin_ks[k][:, C + hc * P: C + (hc + 1) * P],
                            rhs=xin_ks[k][:, s0:s0 + sn],
                            start=(k == 0), stop=(k == KI - 1),
                        )
                    nc.scalar.activation(
                        ht[:, hc, s0:s0 + sn], ps[:, :sn], relu,
                        bias=b1_s[:, hc:hc + 1],
                    )

            # Bridge warmup: keep the PE hot while the first W2 chunk lands.
            for _ in range(12):
                nc.tensor.matmul(wps[:], lhsT=wsrc[:, :64], rhs=wsrc[:],
                                 start=True, stop=True)

            # Stage B: out[st, oc] = H[st] @ W2[:, oc] + b2[oc]
            for oc in range(OC):
                for (s0, sn) in stiles:
                    ps = psumB.tile([P, 512], f32, tag="psB")
                    for hc in range(HC):
                        nc.tensor.matmul(
                            ps[:sn, :],
                            lhsT=ht[:, hc, s0:s0 + sn],
                            rhs=w2_cs[oc][:, hc * 512:(hc + 1) * 512],
                            start=(hc == 0), stop=(hc == HC - 1),
                        )
                    ot = outp.tile([P, 512], f32, tag="ot")
                    nc.vector.tensor_add(
                        out=ot[:sn, :],
                        in0=ps[:sn, :],
                        in1=b2_s[:sn, oc * 512:(oc + 1) * 512],
                    )
                    nc.scalar.dma_start(
                        out[s0:s0 + sn, oc * 512:(oc + 1) * 512], ot[:sn, :]
                    )

    nc.compile()
    return nc


def kernel(X, X_head_idx, W1, b1, W2, b2):
    X = np.ascontiguousarray(np.asarray(X, dtype=np.float32))
    idx = np.asarray(X_head_idx).astype(np.int64)
    W1 = np.asarray(W1, dtype=np.float32)
    b1 = np.asarray(b1, dtype=np.float32)
    W2 = np.asarray(W2, dtype=np.float32)
    b2 = np.asarray(b2, dtype=np.float32)

    batch = X.shape[0]
    counts = np.bincount(idx, minlength=N_HEADS)
    order = np.argsort(idx, kind="stable")
    positions = np.split(order, np.cumsum(counts)[:-1])

    C = max(512, int(-(-counts.max() // 64)) * 64)
    if C not in _NC_CACHE:
        _NC_CACHE[C] = build_nc(C)
    nc = _NC_CACHE[C]

    in_maps = []
    for h in range(N_HEADS):
        pos = positions[h]
        # xin[k, p, :] = [ X[pos, k*128+p] (len C, padded) | W1[h, k*128+p, :] ]
        xin = np.zeros((KI, P, C + HID), dtype=np.float32)
        if len(pos):
            xk = X[pos].T.reshape(KI, P, len(pos))          # [k, p, c]
            xin[:, :, :len(pos)] = xk
        xin[:, :, C:] = W1[h].reshape(KI, P, HID)
        # w2 packed: [oc, p, hc*512 + o'] = W2[h, hc*128 + p, oc*512 + o']
        w2t = np.transpose(W2[h].reshape(HC, P, OUT_F), (1, 0, 2))  # [p, hc, of]
        w2p = np.empty((OC, P, HC * 512), dtype=np.float32)
        for oc in range(OC):
            w2p[oc] = w2t[:, :, oc * 512:(oc + 1) * 512].reshape(P, HC * 512)
        in_maps.append({
            "xin": xin,
            "b1s": np.ascontiguousarray(b1[h].reshape(HC, P).T),
            "w2p": w2p,
            "b2": np.ascontiguousarray(b2[h][None, :]),
        })

    res = run_bass_kernel_spmd(nc, in_maps, list(range(N_CORES)))

    out = np.empty((batch, OUT_F), dtype=np.float32)
    for h in range(N_HEADS):
        pos = positions[h]
        if len(pos):
            out[pos] = res.results[h]["out"][:len(pos)]
    return out


# revision 17
# speedup vs baseline: 1.1668x; 1.0444x over previous
"""MultiHeadDecoder (moe_routing) Trainium2 kernel.

Strategy: expert-parallel. Each of the 8 cores owns one head's weights.
Host groups samples by head index, pads each group to a common capacity C
(multiple of 64), and transposes X so the contraction dim lands on
partitions. Each core runs a dense 2-layer MLP (256->512 relu, 512->2048)
for its head's samples. Host scatters rows back to original order.

Layer 1 computes H^T (hid on partitions) so layer 2 can contract over hid
without an on-chip transpose:
  H^T[hc]  = W1[:, hc].T @ X^T      (lhsT=W1 chunk, rhs=X^T chunk)
  out[st]  = (H^T[:, st]).T @ W2    (lhsT=H^T chunk, rhs=W2 chunk)

Matmuls run in float32r (fp32 bits, full PE rate, tf32-ish multiply).
Inputs are packed host-side into the exact SBUF layout so every DMA has
long contiguous runs per partition. All inputs stream on the sync (SP)
HWDGE ring; all output stores go on the scalar (Act) ring so they never
queue behind the W2 stream (rings are FIFO per issuing engine). Stage B
is ordered oc-outer so only the first W2 chunk's DMA gates its start.
Dummy matmuls keep the PE's HAM clock-gate warm while DMAs stream.
"""

import ml_dtypes
import numpy as np

import concourse.bass as bass
import concourse.mybir as mybir
from concourse import bacc
from concourse.tile import TileContext
from concourse.bass_utils import run_bass_kernel_spmd

IN_F, HID, OUT_F, N_HEADS, BATCH = 256, 512, 2048, 8, 4096
N_CORES = 8
P = 128
KI = IN_F // P     # 2  input-feature chunks
HC = HID // P      # 4  hidden chunks
OC = OUT_F // 512  # 4  output-feature chunks of 512

f32 = mybir.dt.float32
f32r = mybir.dt.float32r  # fp32 bits, PE runs at full (bf16) rate, tf32-ish mul
bf16 = mybir.dt.bfloat16

_NC_CACHE: dict = {}


def build_nc(C: int):
    """Build the per-core Bass program for capacity C (multiple of 64)."""
    KF = C + HID     # free size of one k-part: xt_k then w1_k
    stiles = [(s, min(P, C - s)) for s in range(0, C, P)]
    sgroups = [(s, min(512, C - s)) for s in range(0, C, 512)]

    nc = bacc.Bacc("TRN2", target_bir_lowering=False, debug=False,
                   num_devices=N_CORES)
    xin = nc.dram_tensor("xin", [KI, P, KF], f32r, kind="ExternalInput")
    b1s = nc.dram_tensor("b1s", [P, HC], f32, kind="ExternalInput")
    w2p = nc.dram_tensor("w2p", [OC, P, HC * 512], bf16, kind="ExternalInput")
    b2 = nc.dram_tensor("b2", [1, OUT_F], f32, kind="ExternalInput")
    out = nc.dram_tensor("out", [C, OUT_F], f32, kind="ExternalOutput")

    relu = mybir.ActivationFunctionType.Relu

    with TileContext(nc) as tc:
        with (
            tc.tile_pool(name="const", bufs=1) as const,
            tc.tile_pool(name="psumA", bufs=2, space="PSUM") as psumA,
            tc.tile_pool(name="psumB", bufs=5, space="PSUM") as psumB,
            tc.tile_pool(name="psumW", bufs=1, space="PSUM") as psumW,
            tc.tile_pool(name="outp", bufs=6) as outp,
        ):
            # HAM warmup: dummy matmuls with no DMA deps keep the PE busy
            # while inputs stream in, so real matmuls run at 2.4 GHz.
            wsrc = const.tile([P, 64], f32, tag="warm")
            nc.vector.memset(wsrc[:], 0.0)
            wps = psumW.tile([64, 64], f32, tag="warmps")
            for _ in range(35):
                nc.tensor.matmul(wps[:], lhsT=wsrc[:, :64], rhs=wsrc[:],
                                 start=True, stop=True)

            # Stage-A inputs first so the PE starts ASAP; W2 streams behind.
            # Two k-part DMAs so the k=0 matmuls can start at half-arrival.
            xin_ks = []
            for k in range(KI):
                xk = const.tile([P, KF], f32r, tag=f"xin_{k}")
                nc.sync.dma_start(xk[:], xin[k])
                xin_ks.append(xk)
            b1_s = const.tile([P, HC], f32)
            nc.sync.dma_start(b1_s[:], b1s[:])
            b2_row = const.tile([1, OUT_F], f32)
            nc.sync.dma_start(b2_row[:], b2[:])
            b2_s = const.tile([P, OUT_F], f32)
            nc.gpsimd.partition_broadcast(b2_s[:], b2_row[:])
            w2_cs = []
            for oc in range(OC):
                w2_c = const.tile([P, HC * 512], bf16, tag=f"w2_{oc}")
                nc.sync.dma_start(w2_c[:], w2p[oc])
                w2_cs.append(w2_c)

            # Stage A: H^T [hid(part), sample(free)], relu(x @ W1 + b1)
            # sgroups outer so stage B's early sample tiles are ready sooner.
            ht = const.tile([P, HC, C], bf16)
            for (s0, sn) in sgroups:
                for hc in range(HC):
                    ps = psumA.tile([P, 512], f32, tag="psA")
                    for k in range(KI):
                        nc.tensor.matmul(
                            ps[:, :sn],
                            lhsT=xin_ks[k][:, C + hc * P: C + (hc + 1) * P],
                            rhs=xin_ks[k][:, s0:s0 + sn],
                            start=(k == 0), stop=(k == KI - 1),
                        )
                    nc.scalar.activation(
                        ht[:, hc, s0:s0 + sn], ps[:, :sn], relu,
                        bias=b1_s[:, hc:hc + 1],
                    )

            # Bridge warmup: keep the PE hot while the first W2 chunk lands.
            for _ in range(12):
                nc.tensor.matmul(wps[:], lhsT=wsrc[:, :64], rhs=wsrc[:],
                                 start=True, stop=True)

            # Stage B: out[st, oc] = H[st] @ W2[:, oc] + b2[oc]
            for oc in range(OC):
                for (s0, sn) in stiles:
                    ps = psumB.tile([P, 512], f32, tag="psB")
                    for hc in range(HC):
                        nc.tensor.matmul(
                            ps[:sn, :],
                            lhsT=ht[:, hc, s0:s0 + sn],
                            rhs=w2_cs[oc][:, hc * 512:(hc + 1) * 512],
                            start=(hc == 0), stop=(hc == HC - 1),
                        )
                    ot = outp.tile([P, 512], f32, tag="ot")
                    nc.vector.tensor_add(
                        out=ot[:sn, :],
                        in0=ps[:sn, :],
                        in1=b2_s[:sn, oc * 512:(oc + 1) * 512],
                    )
                    nc.scalar.dma_start(
                        out[s0:s0 + sn, oc * 512:(oc + 1) * 512], ot[:sn, :]
                    )

    nc.compile()
    return nc


def kernel(X, X_head_idx, W1, b1, W2, b2):
    X = np.ascontiguousarray(np.asarray(X, dtype=np.float32))
    idx = np.asarray(X_head_idx).astype(np.int64)
    W1 = np.asarray(W1, dtype=np.float32)
    b1 = np.asarray(b1, dtype=np.float32)
    W2 = np.asarray(W2, dtype=np.float32)
    b2 = np.asarray(b2, dtype=np.float32)

    batch = X.shape[0]
    counts = np.bincount(idx, minlength=N_HEADS)
    order = np.argsort(idx, kind="stable")
    positions = np.split(order, np.cumsum(counts)[:-1])

    C = max(512, int(-(-counts.max() // 64)) * 64)
    if C not in _NC_CACHE:
        _NC_CACHE[C] = build_nc(C)
    nc = _NC_CACHE[C]

    in_maps = []
    for h in range(N_HEADS):
        pos = positions[h]
        # xin[k, p, :] = [ X[pos, k*128+p] (len C, padded) | W1[h, k*128+p, :] ]
        xin = np.zeros((KI, P, C + HID), dtype=np.float32)
        if len(pos):
            xk = X[pos].T.reshape(KI, P, len(pos))          # [k, p, c]
            xin[:, :, :len(pos)] = xk
        xin[:, :, C:] = W1[h].reshape(KI, P, HID)
        # w2 packed: [oc, p, hc*512 + o'] = W2[h, hc*128 + p, oc*512 + o']
        w2t = np.transpose(W2[h].reshape(HC, P, OUT_F), (1, 0, 2))  # [p, hc, of]
        w2p = np.empty((OC, P, HC * 512), dtype=ml_dtypes.bfloat16)
        for oc in range(OC):
            w2p[oc] = w2t[:, :, oc * 512:(oc + 1) * 512].reshape(P, HC * 512)
        in_maps.append({
            "xin": xin,
            "b1s": np.ascontiguousarray(b1[h].reshape(HC, P).T),
            "w2p": w2p,
            "b2": np.ascontiguousarray(b2[h][None, :]),
        })

    res = run_bass_kernel_spmd(nc, in_maps, list(range(N_CORES)))

    out = np.empty((batch, OUT_F), dtype=np.float32)
    for h in range(N_HEADS):
        pos = positions[h]
        if len(pos):
            out[pos] = res.results[h]["out"][:len(pos)]
    return out


# revision 18
# speedup vs baseline: 1.1907x; 1.0205x over previous
"""MultiHeadDecoder (moe_routing) Trainium2 kernel.

Strategy: expert-parallel. Each of the 8 cores owns one head's weights.
Host groups samples by head index, pads each group to a common capacity C
(multiple of 64), and transposes X so the contraction dim lands on
partitions. Each core runs a dense 2-layer MLP (256->512 relu, 512->2048)
for its head's samples. Host scatters rows back to original order.

Layer 1 computes H^T (hid on partitions) so layer 2 can contract over hid
without an on-chip transpose:
  H^T[hc]  = W1[:, hc].T @ X^T      (lhsT=W1 chunk, rhs=X^T chunk)
  out[st]  = (H^T[:, st]).T @ W2    (lhsT=H^T chunk, rhs=W2 chunk)

Matmuls run in float32r (fp32 bits, full PE rate, tf32-ish multiply).
Inputs are packed host-side into the exact SBUF layout so every DMA has
long contiguous runs per partition. All inputs stream on the sync (SP)
HWDGE ring; all output stores go on the scalar (Act) ring so they never
queue behind the W2 stream (rings are FIFO per issuing engine). Stage B
is ordered oc-outer so only the first W2 chunk's DMA gates its start.
Dummy matmuls keep the PE's HAM clock-gate warm while DMAs stream.
"""

import ml_dtypes
import numpy as np

import concourse.bass as bass
import concourse.mybir as mybir
from concourse import bacc
from concourse.tile import TileContext
from concourse.bass_utils import run_bass_kernel_spmd

IN_F, HID, OUT_F, N_HEADS, BATCH = 256, 512, 2048, 8, 4096
N_CORES = 8
P = 128
KI = IN_F // P     # 2  input-feature chunks
HC = HID // P      # 4  hidden chunks
OC = OUT_F // 512  # 4  output-feature chunks of 512

f32 = mybir.dt.float32
f32r = mybir.dt.float32r  # fp32 bits, PE runs at full (bf16) rate, tf32-ish mul
bf16 = mybir.dt.bfloat16

_NC_CACHE: dict = {}


def build_nc(C: int):
    """Build the per-core Bass program for capacity C (multiple of 64)."""
    KF = C + HID     # free size of one k-part: xt_k then w1_k
    stiles = [(s, min(P, C - s)) for s in range(0, C, P)]
    sgroups = [(s, min(512, C - s)) for s in range(0, C, 512)]

    nc = bacc.Bacc("TRN2", target_bir_lowering=False, debug=False,
                   num_devices=N_CORES)
    xin = nc.dram_tensor("xin", [KI, P, KF], bf16, kind="ExternalInput")
    b1s = nc.dram_tensor("b1s", [P, HC], f32, kind="ExternalInput")
    w2p = nc.dram_tensor("w2p", [OC, P, HC * 512], bf16, kind="ExternalInput")
    b2 = nc.dram_tensor("b2", [1, OUT_F], f32, kind="ExternalInput")
    out = nc.dram_tensor("out", [C, OUT_F], f32, kind="ExternalOutput")

    relu = mybir.ActivationFunctionType.Relu

    with TileContext(nc) as tc:
        with (
            tc.tile_pool(name="const", bufs=1) as const,
            tc.tile_pool(name="psumA", bufs=2, space="PSUM") as psumA,
            tc.tile_pool(name="psumB", bufs=5, space="PSUM") as psumB,
            tc.tile_pool(name="psumW", bufs=1, space="PSUM") as psumW,
            tc.tile_pool(name="outp", bufs=6) as outp,
        ):
            # HAM warmup: dummy matmuls with no DMA deps keep the PE busy
            # while inputs stream in, so real matmuls run at 2.4 GHz.
            wsrc = const.tile([P, 64], f32, tag="warm")
            nc.vector.memset(wsrc[:], 0.0)
            wps = psumW.tile([64, 64], f32, tag="warmps")
            for _ in range(16):
                nc.tensor.matmul(wps[:], lhsT=wsrc[:, :64], rhs=wsrc[:],
                                 start=True, stop=True)

            # Stage-A inputs first so the PE starts ASAP; W2 streams behind.
            # Two k-part DMAs so the k=0 matmuls can start at half-arrival.
            xin_ks = []
            for k in range(KI):
                xk = const.tile([P, KF], bf16, tag=f"xin_{k}")
                nc.sync.dma_start(xk[:], xin[k])
                xin_ks.append(xk)
            b1_s = const.tile([P, HC], f32)
            nc.sync.dma_start(b1_s[:], b1s[:])
            b2_row = const.tile([1, OUT_F], f32)
            nc.sync.dma_start(b2_row[:], b2[:])
            b2_s = const.tile([P, OUT_F], f32)
            nc.gpsimd.partition_broadcast(b2_s[:], b2_row[:])
            w2_cs = []
            for oc in range(OC):
                w2_c = const.tile([P, HC * 512], bf16, tag=f"w2_{oc}")
                nc.sync.dma_start(w2_c[:], w2p[oc])
                w2_cs.append(w2_c)

            # Stage A: H^T [hid(part), sample(free)], relu(x @ W1 + b1)
            # sgroups outer so stage B's early sample tiles are ready sooner.
            ht = const.tile([P, HC, C], bf16)
            for (s0, sn) in sgroups:
                for hc in range(HC):
                    ps = psumA.tile([P, 512], f32, tag="psA")
                    for k in range(KI):
                        nc.tensor.matmul(
                            ps[:, :sn],
                            lhsT=xin_ks[k][:, C + hc * P: C + (hc + 1) * P],
                            rhs=xin_ks[k][:, s0:s0 + sn],
                            start=(k == 0), stop=(k == KI - 1),
                        )
                    nc.scalar.activation(
                        ht[:, hc, s0:s0 + sn], ps[:, :sn], relu,
                        bias=b1_s[:, hc:hc + 1],
                    )

            # Bridge warmup: keep the PE hot while the first W2 chunk lands.
            for _ in range(8):
                nc.tensor.matmul(wps[:], lhsT=wsrc[:, :64], rhs=wsrc[:],
                                 start=True, stop=True)

            # Stage B: out[st, oc] = H[st] @ W2[:, oc] + b2[oc]
            for oc in range(OC):
                for (s0, sn) in stiles:
                    ps = psumB.tile([P, 512], f32, tag="psB")
                    for hc in range(HC):
                        nc.tensor.matmul(
                            ps[:sn, :],
                            lhsT=ht[:, hc, s0:s0 + sn],
                            rhs=w2_cs[oc][:, hc * 512:(hc + 1) * 512],
                            start=(hc == 0), stop=(hc == HC - 1),
                        )
                    ot = outp.tile([P, 512], f32, tag="ot")
                    nc.vector.tensor_add(
                        out=ot[:sn, :],
                        in0=ps[:sn, :],
                        in1=b2_s[:sn, oc * 512:(oc + 1) * 512],
                    )
                    nc.scalar.dma_start(
                        out[s0:s0 + sn, oc * 512:(oc + 1) * 512], ot[:sn, :]
                    )

    nc.compile()
    return nc


def kernel(X, X_head_idx, W1, b1, W2, b2):
    X = np.ascontiguousarray(np.asarray(X, dtype=np.float32))
    idx = np.asarray(X_head_idx).astype(np.int64)
    W1 = np.asarray(W1, dtype=np.float32)
    b1 = np.asarray(b1, dtype=np.float32)
    W2 = np.asarray(W2, dtype=np.float32)
    b2 = np.asarray(b2, dtype=np.float32)

    batch = X.shape[0]
    counts = np.bincount(idx, minlength=N_HEADS)
    order = np.argsort(idx, kind="stable")
    positions = np.split(order, np.cumsum(counts)[:-1])

    C = max(512, int(-(-counts.max() // 64)) * 64)
    if C not in _NC_CACHE:
        _NC_CACHE[C] = build_nc(C)
    nc = _NC_CACHE[C]

    in_maps = []
    for h in range(N_HEADS):
        pos = positions[h]
        # xin[k, p, :] = [ X[pos, k*128+p] (len C, padded) | W1[h, k*128+p, :] ]
        xin = np.zeros((KI, P, C + HID), dtype=ml_dtypes.bfloat16)
        if len(pos):
            xk = X[pos].T.reshape(KI, P, len(pos))          # [k, p, c]
            xin[:, :, :len(pos)] = xk.astype(ml_dtypes.bfloat16)
        xin[:, :, C:] = W1[h].reshape(KI, P, HID).astype(ml_dtypes.bfloat16)
        # w2 packed: [oc, p, hc*512 + o'] = W2[h, hc*128 + p, oc*512 + o']
        w2t = np.transpose(W2[h].reshape(HC, P, OUT_F), (1, 0, 2))  # [p, hc, of]
        w2p = np.empty((OC, P, HC * 512), dtype=ml_dtypes.bfloat16)
        for oc in range(OC):
            w2p[oc] = w2t[:, :, oc * 512:(oc + 1) * 512].reshape(P, HC * 512)
        in_maps.append({
            "xin": xin,
            "b1s": np.ascontiguousarray(b1[h].reshape(HC, P).T),
            "w2p": w2p,
            "b2": np.ascontiguousarray(b2[h][None, :]),
        })

    res = run_bass_kernel_spmd(nc, in_maps, list(range(N_CORES)))

    out = np.empty((batch, OUT_F), dtype=np.float32)
    for h in range(N_HEADS):
        pos = positions[h]
        if len(pos):
            out[pos] = res.results[h]["out"][:len(pos)]
    return out


# revision 19
# speedup vs baseline: 1.1944x; 1.0031x over previous
"""MultiHeadDecoder (moe_routing) Trainium2 kernel.

Strategy: expert-parallel. Each of the 8 cores owns one head's weights.
Host groups samples by head index, pads each group to a common capacity C
(multiple of 64), and transposes X so the contraction dim lands on
partitions. Each core runs a dense 2-layer MLP (256->512 relu, 512->2048)
for its head's samples. Host scatters rows back to original order.

Layer 1 computes H^T (hid on partitions) so layer 2 can contract over hid
without an on-chip transpose:
  H^T[hc]  = W1[:, hc].T @ X^T      (lhsT=W1 chunk, rhs=X^T chunk)
  out[st]  = (H^T[:, st]).T @ W2    (lhsT=H^T chunk, rhs=W2 chunk)

Matmuls run in float32r (fp32 bits, full PE rate, tf32-ish multiply).
Inputs are packed host-side into the exact SBUF layout so every DMA has
long contiguous runs per partition. All inputs stream on the sync (SP)
HWDGE ring; all output stores go on the scalar (Act) ring so they never
queue behind the W2 stream (rings are FIFO per issuing engine). Stage B
is ordered oc-outer so only the first W2 chunk's DMA gates its start.
Dummy matmuls keep the PE's HAM clock-gate warm while DMAs stream.
"""

import ml_dtypes
import numpy as np

import concourse.bass as bass
import concourse.mybir as mybir
from concourse import bacc
from concourse.tile import TileContext
from concourse.bass_utils import run_bass_kernel_spmd

IN_F, HID, OUT_F, N_HEADS, BATCH = 256, 512, 2048, 8, 4096
N_CORES = 8
P = 128
KI = IN_F // P     # 2  input-feature chunks
HC = HID // P      # 4  hidden chunks
OC = OUT_F // 512  # 4  output-feature chunks of 512

f32 = mybir.dt.float32
f32r = mybir.dt.float32r  # fp32 bits, PE runs at full (bf16) rate, tf32-ish mul
bf16 = mybir.dt.bfloat16

_NC_CACHE: dict = {}


def build_nc(C: int):
    """Build the per-core Bass program for capacity C (multiple of 64)."""
    KF = C + HID     # free size of one k-part: xt_k then w1_k
    stiles = [(s, min(P, C - s)) for s in range(0, C, P)]
    sgroups = [(s, min(512, C - s)) for s in range(0, C, 512)]

    nc = bacc.Bacc("TRN2", target_bir_lowering=False, debug=False,
                   num_devices=N_CORES)
    xin = nc.dram_tensor("xin", [KI, P, KF], bf16, kind="ExternalInput")
    b1s = nc.dram_tensor("b1s", [P, HC], f32, kind="ExternalInput")
    w2p = nc.dram_tensor("w2p", [OC, P, HC * 512], bf16, kind="ExternalInput")
    b2 = nc.dram_tensor("b2", [1, OUT_F], f32, kind="ExternalInput")
    out = nc.dram_tensor("out", [C, OUT_F], bf16, kind="ExternalOutput")

    relu = mybir.ActivationFunctionType.Relu

    with TileContext(nc) as tc:
        with (
            tc.tile_pool(name="const", bufs=1) as const,
            tc.tile_pool(name="psumA", bufs=2, space="PSUM") as psumA,
            tc.tile_pool(name="psumB", bufs=5, space="PSUM") as psumB,
            tc.tile_pool(name="psumW", bufs=1, space="PSUM") as psumW,
            tc.tile_pool(name="outp", bufs=6) as outp,
        ):
            # HAM warmup: dummy matmuls with no DMA deps keep the PE busy
            # while inputs stream in, so real matmuls run at 2.4 GHz.
            wsrc = const.tile([P, 64], f32, tag="warm")
            nc.vector.memset(wsrc[:], 0.0)
            wps = psumW.tile([64, 64], f32, tag="warmps")
            for _ in range(16):
                nc.tensor.matmul(wps[:], lhsT=wsrc[:, :64], rhs=wsrc[:],
                                 start=True, stop=True)

            # Stage-A inputs first so the PE starts ASAP; W2 streams behind.
            # Two k-part DMAs so the k=0 matmuls can start at half-arrival.
            xin_ks = []
            for k in range(KI):
                xk = const.tile([P, KF], bf16, tag=f"xin_{k}")
                nc.sync.dma_start(xk[:], xin[k])
                xin_ks.append(xk)
            b1_s = const.tile([P, HC], f32)
            nc.sync.dma_start(b1_s[:], b1s[:])
            b2_row = const.tile([1, OUT_F], f32)
            nc.sync.dma_start(b2_row[:], b2[:])
            b2_s = const.tile([P, OUT_F], f32)
            nc.gpsimd.partition_broadcast(b2_s[:], b2_row[:])
            w2_cs = []
            for oc in range(OC):
                w2_c = const.tile([P, HC * 512], bf16, tag=f"w2_{oc}")
                nc.sync.dma_start(w2_c[:], w2p[oc])
                w2_cs.append(w2_c)

            # Stage A: H^T [hid(part), sample(free)], relu(x @ W1 + b1)
            # sgroups outer so stage B's early sample tiles are ready sooner.
            ht = const.tile([P, HC, C], bf16)
            for (s0, sn) in sgroups:
                for hc in range(HC):
                    ps = psumA.tile([P, 512], f32, tag="psA")
                    for k in range(KI):
                        nc.tensor.matmul(
                            ps[:, :sn],
                            lhsT=xin_ks[k][:, C + hc * P: C + (hc + 1) * P],
                            rhs=xin_ks[k][:, s0:s0 + sn],
                            start=(k == 0), stop=(k == KI - 1),
                        )
                    nc.scalar.activation(
                        ht[:, hc, s0:s0 + sn], ps[:, :sn], relu,
                        bias=b1_s[:, hc:hc + 1],
                    )

            # Bridge warmup: keep the PE hot while the first W2 chunk lands.
            for _ in range(8):
                nc.tensor.matmul(wps[:], lhsT=wsrc[:, :64], rhs=wsrc[:],
                                 start=True, stop=True)

            # Stage B: out[st, oc] = H[st] @ W2[:, oc] + b2[oc]
            for oc in range(OC):
                for (s0, sn) in stiles:
                    ps = psumB.tile([P, 512], f32, tag="psB")
                    for hc in range(HC):
                        nc.tensor.matmul(
                            ps[:sn, :],
                            lhsT=ht[:, hc, s0:s0 + sn],
                            rhs=w2_cs[oc][:, hc * 512:(hc + 1) * 512],
                            start=(hc == 0), stop=(hc == HC - 1),
                        )
                    ot = outp.tile([P, 512], bf16, tag="ot")
                    nc.vector.tensor_add(
                        out=ot[:sn, :],
                        in0=ps[:sn, :],
                        in1=b2_s[:sn, oc * 512:(oc + 1) * 512],
                    )
                    nc.scalar.dma_start(
                        out[s0:s0 + sn, oc * 512:(oc + 1) * 512], ot[:sn, :]
                    )

    nc.compile()
    return nc


def kernel(X, X_head_idx, W1, b1, W2, b2):
    X = np.ascontiguousarray(np.asarray(X, dtype=np.float32))
    idx = np.asarray(X_head_idx).astype(np.int64)
    W1 = np.asarray(W1, dtype=np.float32)
    b1 = np.asarray(b1, dtype=np.float32)
    W2 = np.asarray(W2, dtype=np.float32)
    b2 = np.asarray(b2, dtype=np.float32)

    batch = X.shape[0]
    counts = np.bincount(idx, minlength=N_HEADS)
    order = np.argsort(idx, kind="stable")
    positions = np.split(order, np.cumsum(counts)[:-1])

    C = max(512, int(-(-counts.max() // 64)) * 64)
    if C not in _NC_CACHE:
        _NC_CACHE[C] = build_nc(C)
    nc = _NC_CACHE[C]

    in_maps = []
    for h in range(N_HEADS):
        pos = positions[h]
        # xin[k, p, :] = [ X[pos, k*128+p] (len C, padded) | W1[h, k*128+p, :] ]
        xin = np.zeros((KI, P, C + HID), dtype=ml_dtypes.bfloat16)
        if len(pos):
            xk = X[pos].T.reshape(KI, P, len(pos))          # [k, p, c]
            xin[:, :, :len(pos)] = xk.astype(ml_dtypes.bfloat16)
        xin[:, :, C:] = W1[h].reshape(KI, P, HID).astype(ml_dtypes.bfloat16)
        # w2 packed: [oc, p, hc*512 + o'] = W2[h, hc*128 + p, oc*512 + o']
        w2t = np.transpose(W2[h].reshape(HC, P, OUT_F), (1, 0, 2))  # [p, hc, of]
        w2p = np.empty((OC, P, HC * 512), dtype=ml_dtypes.bfloat16)
        for oc in range(OC):
            w2p[oc] = w2t[:, :, oc * 512:(oc + 1) * 512].reshape(P, HC * 512)
        in_maps.append({
            "xin": xin,
            "b1s": np.ascontiguousarray(b1[h].reshape(HC, P).T),
            "w2p": w2p,
            "b2": np.ascontiguousarray(b2[h][None, :]),
        })

    res = run_bass_kernel_spmd(nc, in_maps, list(range(N_CORES)))

    out = np.empty((batch, OUT_F), dtype=np.float32)
    for h in range(N_HEADS):
        pos = positions[h]
        if len(pos):
            out[pos] = res.results[h]["out"][:len(pos)].astype(np.float32)
    return out


# revision 20
# speedup vs baseline: 1.2563x; 1.0518x over previous
"""MultiHeadDecoder (moe_routing) Trainium2 kernel.

Strategy: expert-parallel. Each of the 8 cores owns one head's weights.
Host groups samples by head index, pads each group to a common capacity C
(multiple of 64), and transposes X so the contraction dim lands on
partitions. Each core runs a dense 2-layer MLP (256->512 relu, 512->2048)
for its head's samples. Host scatters rows back to original order.

Layer 1 computes H^T (hid on partitions) so layer 2 can contract over hid
without an on-chip transpose:
  H^T[hc]  = W1[:, hc].T @ X^T      (lhsT=W1 chunk, rhs=X^T chunk)
  out[st]  = (H^T[:, st]).T @ W2    (lhsT=H^T chunk, rhs=W2 chunk)

Matmuls run in float32r (fp32 bits, full PE rate, tf32-ish multiply).
Inputs are packed host-side into the exact SBUF layout so every DMA has
long contiguous runs per partition. All inputs stream on the sync (SP)
HWDGE ring; all output stores go on the scalar (Act) ring so they never
queue behind the W2 stream (rings are FIFO per issuing engine). Stage B
is ordered oc-outer so only the first W2 chunk's DMA gates its start.
Dummy matmuls keep the PE's HAM clock-gate warm while DMAs stream.
"""

import ml_dtypes
import numpy as np

import concourse.bass as bass
import concourse.mybir as mybir
from concourse import bacc
from concourse.tile import TileContext
from concourse.bass_utils import run_bass_kernel_spmd

IN_F, HID, OUT_F, N_HEADS, BATCH = 256, 512, 2048, 8, 4096
N_CORES = 8
P = 128
KI = IN_F // P     # 2  input-feature chunks
HC = HID // P      # 4  hidden chunks
OC = OUT_F // 512  # 4  output-feature chunks of 512

f32 = mybir.dt.float32
f32r = mybir.dt.float32r  # fp32 bits, PE runs at full (bf16) rate, tf32-ish mul
bf16 = mybir.dt.bfloat16

_NC_CACHE: dict = {}


def build_nc(C: int):
    """Build the per-core Bass program for capacity C (multiple of 64)."""
    KF = C + HID     # free size of one k-part: xt_k then w1_k
    stiles = [(s, min(P, C - s)) for s in range(0, C, P)]
    sgroups = [(s, min(512, C - s)) for s in range(0, C, 512)]

    nc = bacc.Bacc("TRN2", target_bir_lowering=False, debug=False,
                   num_devices=N_CORES)
    xin = nc.dram_tensor("xin", [KI, P, KF], bf16, kind="ExternalInput")
    b1s = nc.dram_tensor("b1s", [P, HC], f32, kind="ExternalInput")
    w2p = nc.dram_tensor("w2p", [OC, P, HC * 512], bf16, kind="ExternalInput")
    b2 = nc.dram_tensor("b2", [1, OUT_F], f32, kind="ExternalInput")
    out = nc.dram_tensor("out", [C, OUT_F], bf16, kind="ExternalOutput")

    relu = mybir.ActivationFunctionType.Relu

    with TileContext(nc) as tc:
        with (
            tc.tile_pool(name="const", bufs=1) as const,
            tc.tile_pool(name="psumA", bufs=2, space="PSUM") as psumA,
            tc.tile_pool(name="psumB", bufs=5, space="PSUM") as psumB,
            tc.tile_pool(name="psumW", bufs=1, space="PSUM") as psumW,
            tc.tile_pool(name="outp", bufs=6) as outp,
        ):
            # HAM warmup: dummy matmuls with no DMA deps keep the PE busy
            # while inputs stream in, so real matmuls run at 2.4 GHz.
            wsrc = const.tile([P, 64], f32, tag="warm")
            nc.vector.memset(wsrc[:], 0.0)
            wps = psumW.tile([64, 64], f32, tag="warmps")
            for _ in range(16):
                nc.tensor.matmul(wps[:], lhsT=wsrc[:, :64], rhs=wsrc[:],
                                 start=True, stop=True)

            # Stage-A inputs first so the PE starts ASAP; W2 streams behind.
            # Two k-part DMAs so the k=0 matmuls can start at half-arrival.
            xin_ks = []
            for k in range(KI):
                xk = const.tile([P, KF], bf16, tag=f"xin_{k}")
                nc.sync.dma_start(xk[:], xin[k])
                xin_ks.append(xk)
            b1_s = const.tile([P, HC], f32)
            nc.sync.dma_start(b1_s[:], b1s[:])
            b2_row = const.tile([1, OUT_F], f32)
            nc.sync.dma_start(b2_row[:], b2[:])
            b2_s = const.tile([P, OUT_F], f32)
            nc.gpsimd.partition_broadcast(b2_s[:], b2_row[:])
            w2_cs = []
            for oc in range(OC):
                w2_c = const.tile([P, HC * 512], bf16, tag=f"w2_{oc}")
                nc.sync.dma_start(w2_c[:], w2p[oc])
                w2_cs.append(w2_c)

            # Stage A: H^T [hid(part), sample(free)], relu(x @ W1 + b1)
            # sgroups outer so stage B's early sample tiles are ready sooner.
            ht = const.tile([P, HC, C], bf16)
            for (s0, sn) in sgroups:
                for hc in range(HC):
                    ps = psumA.tile([P, 512], f32, tag="psA")
                    for k in range(KI):
                        nc.tensor.matmul(
                            ps[:, :sn],
                            lhsT=xin_ks[k][:, C + hc * P: C + (hc + 1) * P],
                            rhs=xin_ks[k][:, s0:s0 + sn],
                            start=(k == 0), stop=(k == KI - 1),
                        )
                    nc.scalar.activation(
                        ht[:, hc, s0:s0 + sn], ps[:, :sn], relu,
                        bias=b1_s[:, hc:hc + 1],
                    )

            # Bridge warmup: keep the PE hot while the first W2 chunk lands.
            for _ in range(6):
                nc.tensor.matmul(wps[:], lhsT=wsrc[:, :64], rhs=wsrc[:],
                                 start=True, stop=True)

            # Stage B: out[st, oc] = H[st] @ W2[:, oc] + b2[oc]
            for oc in range(OC):
                for (s0, sn) in stiles:
                    ps = psumB.tile([P, 512], f32, tag="psB")
                    for hc in range(HC):
                        nc.tensor.matmul(
                            ps[:sn, :],
                            lhsT=ht[:, hc, s0:s0 + sn],
                            rhs=w2_cs[oc][:, hc * 512:(hc + 1) * 512],
                            start=(hc == 0), stop=(hc == HC - 1),
                        )
                    ot = outp.tile([P, 512], bf16, tag="ot")
                    nc.vector.tensor_add(
                        out=ot[:sn, :],
                        in0=ps[:sn, :],
                        in1=b2_s[:sn, oc * 512:(oc + 1) * 512],
                    )
                    # oc 0 stores overlap the input stream (keep off the
                    # sync ring); later ones alternate rings to drain 2x.
                    seng = nc.scalar if (oc == 0 or (s0 // P) % 2 == 0) else nc.sync
                    seng.dma_start(
                        out[s0:s0 + sn, oc * 512:(oc + 1) * 512], ot[:sn, :]
                    )

    nc.compile()
    return nc


def kernel(X, X_head_idx, W1, b1, W2, b2):
    X = np.ascontiguousarray(np.asarray(X, dtype=np.float32))
    idx = np.asarray(X_head_idx).astype(np.int64)
    W1 = np.asarray(W1, dtype=np.float32)
    b1 = np.asarray(b1, dtype=np.float32)
    W2 = np.asarray(W2, dtype=np.float32)
    b2 = np.asarray(b2, dtype=np.float32)

    batch = X.shape[0]
    counts = np.bincount(idx, minlength=N_HEADS)
    order = np.argsort(idx, kind="stable")
    positions = np.split(order, np.cumsum(counts)[:-1])

    C = max(512, int(-(-counts.max() // 64)) * 64)
    if C not in _NC_CACHE:
        _NC_CACHE[C] = build_nc(C)
    nc = _NC_CACHE[C]

    in_maps = []
    for h in range(N_HEADS):
        pos = positions[h]
        # xin[k, p, :] = [ X[pos, k*128+p] (len C, padded) | W1[h, k*128+p, :] ]
        xin = np.zeros((KI, P, C + HID), dtype=ml_dtypes.bfloat16)
        if len(pos):
            xk = X[pos].T.reshape(KI, P, len(pos))          # [k, p, c]
            xin[:, :, :len(pos)] = xk.astype(ml_dtypes.bfloat16)
        xin[:, :, C:] = W1[h].reshape(KI, P, HID).astype(ml_dtypes.bfloat16)
        # w2 packed: [oc, p, hc*512 + o'] = W2[h, hc*128 + p, oc*512 + o']
        w2t = np.transpose(W2[h].reshape(HC, P, OUT_F), (1, 0, 2))  # [p, hc, of]
        w2p = np.empty((OC, P, HC * 512), dtype=ml_dtypes.bfloat16)
        for oc in range(OC):
            w2p[oc] = w2t[:, :, oc * 512:(oc + 1) * 512].reshape(P, HC * 512)
        in_maps.append({
            "xin": xin,
            "b1s": np.ascontiguousarray(b1[h].reshape(HC, P).T),
            "w2p": w2p,
            "b2": np.ascontiguousarray(b2[h][None, :]),
        })

    res = run_bass_kernel_spmd(nc, in_maps, list(range(N_CORES)))

    out = np.empty((batch, OUT_F), dtype=np.float32)
    for h in range(N_HEADS):
        pos = positions[h]
        if len(pos):
            out[pos] = res.results[h]["out"][:len(pos)].astype(np.float32)
    return out


# revision 22
# speedup vs baseline: 1.2705x; 1.0113x over previous
"""MultiHeadDecoder (moe_routing) Trainium2 kernel.

Strategy: expert-parallel. Each of the 8 cores owns one head's weights.
Host groups samples by head index, pads each group to a common capacity C
(multiple of 64), and transposes X so the contraction dim lands on
partitions. Each core runs a dense 2-layer MLP (256->512 relu, 512->2048)
for its head's samples. Host scatters rows back to original order.

Layer 1 computes H^T (hid on partitions) so layer 2 can contract over hid
without an on-chip transpose:
  H^T[hc]  = W1[:, hc].T @ X^T      (lhsT=W1 chunk, rhs=X^T chunk)
  out[st]  = (H^T[:, st]).T @ W2    (lhsT=H^T chunk, rhs=W2 chunk)

Matmuls run in float32r (fp32 bits, full PE rate, tf32-ish multiply).
Inputs are packed host-side into the exact SBUF layout so every DMA has
long contiguous runs per partition. All inputs stream on the sync (SP)
HWDGE ring; all output stores go on the scalar (Act) ring so they never
queue behind the W2 stream (rings are FIFO per issuing engine). Stage B
is ordered oc-outer so only the first W2 chunk's DMA gates its start.
Dummy matmuls keep the PE's HAM clock-gate warm while DMAs stream.
"""

import ml_dtypes
import numpy as np

import concourse.bass as bass
import concourse.mybir as mybir
from concourse import bacc
from concourse.tile import TileContext
from concourse.bass_utils import run_bass_kernel_spmd

IN_F, HID, OUT_F, N_HEADS, BATCH = 256, 512, 2048, 8, 4096
N_CORES = 8
P = 128
KI = IN_F // P     # 2  input-feature chunks
HC = HID // P      # 4  hidden chunks
OC = OUT_F // 512  # 4  output-feature chunks of 512

f32 = mybir.dt.float32
f32r = mybir.dt.float32r  # fp32 bits, PE runs at full (bf16) rate, tf32-ish mul
bf16 = mybir.dt.bfloat16

_NC_CACHE: dict = {}


def build_nc(C: int):
    """Build the per-core Bass program for capacity C (multiple of 64)."""
    KF = C + HID     # free size of one k-part: xt_k then w1_k
    stiles = [(s, min(P, C - s)) for s in range(0, C, P)]
    sgroups = [(s, min(512, C - s)) for s in range(0, C, 512)]

    nc = bacc.Bacc("TRN2", target_bir_lowering=False, debug=False,
                   num_devices=N_CORES)
    xin = nc.dram_tensor("xin", [KI, P, KF], bf16, kind="ExternalInput")
    b1s = nc.dram_tensor("b1s", [P, HC], f32, kind="ExternalInput")
    w2p = nc.dram_tensor("w2p", [OC, P, HC * 512], bf16, kind="ExternalInput")
    b2 = nc.dram_tensor("b2", [1, OUT_F], f32, kind="ExternalInput")
    out = nc.dram_tensor("out", [C, OUT_F], bf16, kind="ExternalOutput")

    relu = mybir.ActivationFunctionType.Relu

    with TileContext(nc) as tc:
        with (
            tc.tile_pool(name="const", bufs=1) as const,
            tc.tile_pool(name="psumA", bufs=4, space="PSUM") as psumA,
            tc.tile_pool(name="psumB", bufs=3, space="PSUM") as psumB,
            tc.tile_pool(name="psumW", bufs=1, space="PSUM") as psumW,
            tc.tile_pool(name="outp", bufs=6) as outp,
        ):
            # HAM warmup: dummy matmuls with no DMA deps keep the PE busy
            # while inputs stream in, so real matmuls run at 2.4 GHz.
            wsrc = const.tile([P, 64], f32, tag="warm")
            nc.vector.memset(wsrc[:], 0.0)
            wps = psumW.tile([64, 64], f32, tag="warmps")
            for _ in range(16):
                nc.tensor.matmul(wps[:], lhsT=wsrc[:, :64], rhs=wsrc[:],
                                 start=True, stop=True)

            # Stage-A inputs first so the PE starts ASAP; W2 streams behind.
            # Two k-part DMAs so the k=0 matmuls can start at half-arrival.
            xin_ks = []
            for k in range(KI):
                xk = const.tile([P, KF], bf16, tag=f"xin_{k}")
                nc.sync.dma_start(xk[:], xin[k])
                xin_ks.append(xk)
            b1_s = const.tile([P, HC], f32)
            nc.sync.dma_start(b1_s[:], b1s[:])
            b2_row = const.tile([1, OUT_F], f32)
            nc.sync.dma_start(b2_row[:], b2[:])
            b2_s = const.tile([P, OUT_F], f32)
            nc.gpsimd.partition_broadcast(b2_s[:], b2_row[:])
            w2_cs = []
            for oc in range(OC):
                w2_c = const.tile([P, HC * 512], bf16, tag=f"w2_{oc}")
                nc.sync.dma_start(w2_c[:], w2p[oc])
                w2_cs.append(w2_c)

            # Stage A: H^T [hid(part), sample(free)], relu(x @ W1 + b1)
            # sgroups outer so stage B's early sample tiles are ready sooner.
            ht = const.tile([P, HC, C], bf16)
            for (s0, sn) in sgroups:
                pss = [psumA.tile([P, 512], f32, tag="psA", name=f"psA{i}") for i in range(HC)]
                # k outer: all k=0 matmuls need only the first xin DMA
                for k in range(KI):
                    for hc in range(HC):
                        nc.tensor.matmul(
                            pss[hc][:, :sn],
                            lhsT=xin_ks[k][:, C + hc * P: C + (hc + 1) * P],
                            rhs=xin_ks[k][:, s0:s0 + sn],
                            start=(k == 0), stop=(k == KI - 1),
                        )
                for hc in range(HC):
                    nc.scalar.activation(
                        ht[:, hc, s0:s0 + sn], pss[hc][:, :sn], relu,
                        bias=b1_s[:, hc:hc + 1],
                    )

            # Stage B: out[st, oc] = H[st] @ W2[:, oc] + b2[oc]
            for oc in range(OC):
                for (s0, sn) in stiles:
                    ps = psumB.tile([P, 512], f32, tag="psB")
                    for hc in range(HC):
                        nc.tensor.matmul(
                            ps[:sn, :],
                            lhsT=ht[:, hc, s0:s0 + sn],
                            rhs=w2_cs[oc][:, hc * 512:(hc + 1) * 512],
                            start=(hc == 0), stop=(hc == HC - 1),
                        )
                    ot = outp.tile([P, 512], bf16, tag="ot")
                    nc.vector.tensor_add(
                        out=ot[:sn, :],
                        in0=ps[:sn, :],
                        in1=b2_s[:sn, oc * 512:(oc + 1) * 512],
                    )
                    # oc 0 stores overlap the input stream (keep off the
                    # sync ring); later ones alternate rings to drain 2x.
                    seng = nc.scalar if (oc == 0 or (s0 // P) % 2 == 0) else nc.sync
                    seng.dma_start(
                        out[s0:s0 + sn, oc * 512:(oc + 1) * 512], ot[:sn, :]
                    )

    nc.compile()
    return nc


def kernel(X, X_head_idx, W1, b1, W2, b2):
    X = np.ascontiguousarray(np.asarray(X, dtype=np.float32))
    idx = np.asarray(X_head_idx).astype(np.int64)
    W1 = np.asarray(W1, dtype=np.float32)
    b1 = np.asarray(b1, dtype=np.float32)
    W2 = np.asarray(W2, dtype=np.float32)
    b2 = np.asarray(b2, dtype=np.float32)

    batch = X.shape[0]
    counts = np.bincount(idx, minlength=N_HEADS)
    order = np.argsort(idx, kind="stable")
    positions = np.split(order, np.cumsum(counts)[:-1])

    C = max(512, int(-(-counts.max() // 64)) * 64)
    if C not in _NC_CACHE:
        _NC_CACHE[C] = build_nc(C)
    nc = _NC_CACHE[C]

    in_maps = []
    for h in range(N_HEADS):
        pos = positions[h]
        # xin[k, p, :] = [ X[pos, k*128+p] (len C, padded) | W1[h, k*128+p, :] ]
        xin = np.zeros((KI, P, C + HID), dtype=ml_dtypes.bfloat16)
        if len(pos):
            xk = X[pos].T.reshape(KI, P, len(pos))          # [k, p, c]
            xin[:, :, :len(pos)] = xk.astype(ml_dtypes.bfloat16)
        xin[:, :, C:] = W1[h].reshape(KI, P, HID).astype(ml_dtypes.bfloat16)
        # w2 packed: [oc, p, hc*512 + o'] = W2[h, hc*128 + p, oc*512 + o']
        w2t = np.transpose(W2[h].reshape(HC, P, OUT_F), (1, 0, 2))  # [p, hc, of]
        w2p = np.empty((OC, P, HC * 512), dtype=ml_dtypes.bfloat16)
        for oc in range(OC):
            w2p[oc] = w2t[:, :, oc * 512:(oc + 1) * 512].reshape(P, HC * 512)
        in_maps.append({
            "xin": xin,
            "b1s": np.ascontiguousarray(b1[h].reshape(HC, P).T),
            "w2p": w2p,
            "b2": np.ascontiguousarray(b2[h][None, :]),
        })

    res = run_bass_kernel_spmd(nc, in_maps, list(range(N_CORES)))

    out = np.empty((batch, OUT_F), dtype=np.float32)
    for h in range(N_HEADS):
        pos = positions[h]
        if len(pos):
            out[pos] = res.results[h]["out"][:len(pos)].astype(np.float32)
    return out
